# revision 13
# baseline (speedup 1.0000x reference)
"""Trainium2 Bass kernel for DilatedSpatialAttention (v2).

B=16, H=W=32, C=256, heads=8, head_dim=32; depthwise 3x3 dilated conv
(SAME) on key/value, then softmax attention per (batch, head) over
S=1024. Data-parallel over batch across 8 cores (2 batches/core).

Design notes (HW-validated):
- Depthwise conv on the Vector engine: 9 per-partition-scalar FMAs over
  shifted views of a zero-padded c-major [128, 36, 36] bf16 tile.
- All PSUM tiles fp32; scores via row-tiled (K=32) matmuls, 2 heads
  packed; P = exp(scale*scores) on ScalarE; AV accumulates [V|1]^T P so
  row 32 carries the softmax denominator; output normalized with a
  stride-0 reciprocal broadcast and DMA'd out.
- Pipeline: per-batch input DMAs issue up front, AV emission deferred
  two units, next-batch prep interleaved as fillers.
"""

import numpy as np

B, H, W, C = 16, 32, 32, 256
HEADS = 8
HD = C // HEADS            # 32
KSZ, DIL = 3, 2
SCALE = float(HD) ** -0.5
NCORES = 8
BPC = B // NCORES          # batches per core
S = H * W                  # 1024
NKT = S // 128             # 8 k/s tiles
AV_DEFER = 2               # units between scores and their AV emission
URG_U = 2                  # units that absorb the current batch's v-path

_CACHE = {}


def _build(nc, tile, bass, mybir, repeat=None, parts="all"):
    from contextlib import ExitStack
    from concourse.masks import make_identity

    f32 = mybir.dt.float32
    bf16 = mybir.dt.bfloat16

    q_d = nc.dram_tensor("query", [BPC, S, C], f32, kind="ExternalInput")
    k_d = nc.dram_tensor("key_in", [BPC, S, C], f32, kind="ExternalInput")
    v_d = nc.dram_tensor("value", [BPC, S, C], f32, kind="ExternalInput")
    ck_d = nc.dram_tensor("conv_kernel", [KSZ * KSZ, C], f32, kind="ExternalInput")
    cb_d = nc.dram_tensor("conv_bias", [C], f32, kind="ExternalInput")
    out_d = nc.dram_tensor("out", [BPC, S, C], f32, kind="ExternalOutput")

    HP = H + 2 * DIL  # padded rows: 36
    WP = W + 2 * DIL  # padded cols: 36

    with ExitStack() as ctx:
        tc = ctx.enter_context(tile.TileContext(nc))
        const = ctx.enter_context(tc.tile_pool(name="const", bufs=1))
        sin = ctx.enter_context(tc.tile_pool(name="sin", bufs=32))
        xpad_p = ctx.enter_context(tc.tile_pool(name="xpad", bufs=8))
        qc_p = ctx.enter_context(tc.tile_pool(name="qc", bufs=4))
        kc_p = ctx.enter_context(tc.tile_pool(name="kc", bufs=4))
        vc_p = ctx.enter_context(tc.tile_pool(name="vc", bufs=4))
        vaug_p = ctx.enter_context(tc.tile_pool(name="vaug", bufs=16))
        p_p = ctx.enter_context(tc.tile_pool(name="pp", bufs=40))
        ot_p = ctx.enter_context(tc.tile_pool(name="ot", bufs=4))
        orow_p = ctx.enter_context(tc.tile_pool(name="orow", bufs=16))
        small_p = ctx.enter_context(tc.tile_pool(name="small", bufs=8))
        # PSUM: trans pool (2 banks) + scores (2x2 banks) + accum (2 banks)
        ppp = ctx.enter_context(tc.tile_pool(name="ppp", bufs=2, space="PSUM"))
        sc_p = ctx.enter_context(tc.tile_pool(name="scp", bufs=2, space="PSUM"))
        acc_p = ctx.enter_context(tc.tile_pool(name="accp", bufs=2, space="PSUM"))

        # ---- constants ----
        ident = const.tile([128, 128], f32)
        make_identity(nc, ident[:])

        # conv weights as per-partition scalars: wcol[c, half, tap]
        wcol = const.tile([128, 2, KSZ * KSZ], f32)
        for half in range(2):
            nc.gpsimd.dma_start(
                out=wcol[:, half],
                in_=bass.AP(ck_d, half * 128, [[1, 128], [C, KSZ * KSZ]]),
            )

        bias_c = const.tile([128, 2], f32)
        for half in range(2):
            nc.gpsimd.dma_start(
                out=bias_c[:, half:half + 1],
                in_=bass.AP(cb_d, half * 128, [[1, 128], [1, 1]]),
            )

        rep_ctx = tc.For_i(0, repeat, 1) if repeat else None
        if rep_ctx is not None:
            ctx.enter_context(rep_ctx)

        state = {}

        def prep_chunks(b):
            """Emit-able closures for batch b's prep; fills state[b]."""
            qc = [qc_p.tile([128, S], bf16, tag="qc", name="qc") for _ in range(2)]
            kc = [kc_p.tile([128, S], bf16, tag="kc", name="kc") for _ in range(2)]
            vc = [vc_p.tile([128, S], f32, tag="vc", name="vc") for _ in range(2)]
            vaug = [vaug_p.tile([128, HEADS * (HD + 1)], bf16, tag="va",
                                name="va") for _ in range(NKT)]
            xpad = {}
            state[b] = (qc, kc, vaug)
            chunks = []

            def mk_xpad():
                for tname in ("k", "v"):
                    for half in range(2):
                        xp = xpad_p.tile([128, HP, WP], bf16, tag="xpad",
                                         name="xpad")
                        # zero only the padding ring; interior is overwritten
                        eng = nc.gpsimd
                        eng.memset(xp[:, 0:DIL, :], 0.0)
                        eng.memset(xp[:, HP - DIL:HP, :], 0.0)
                        eng.memset(xp[:, DIL:HP - DIL, 0:DIL], 0.0)
                        eng.memset(xp[:, DIL:HP - DIL, WP - DIL:WP], 0.0)
                        xpad[(tname, half)] = xp
            chunks.append(mk_xpad)

            st_tiles = {}

            def mk_dma(tname, dram, kt):
                def go():
                    st = sin.tile([128, C], f32, tag="sin", name="st")
                    nc.sync.dma_start(
                        out=st[:], in_=dram[b, kt * 128:(kt + 1) * 128, :])
                    st_tiles[(tname, kt)] = st
                return go

            def mk_trans(tname, kt):
                def go():
                    st = st_tiles[(tname, kt)]
                    for half in range(2):
                        pt = ppp.tile([128, 512], f32, tag="pp", name="pt")
                        nc.tensor.transpose(
                            pt[:, 0:128], st[:, 128 * half:128 * half + 128],
                            ident[:])
                        if tname == "q":
                            nc.vector.tensor_copy(
                                out=qc[half][:, kt * 128:(kt + 1) * 128],
                                in_=pt[:, 0:128])
                        else:
                            dst = xpad[(tname, half)][
                                :, DIL + 4 * kt:DIL + 4 * kt + 4, DIL:DIL + W]
                            nc.vector.tensor_copy(
                                out=dst,
                                in_=pt[:, 0:128].rearrange(
                                    "p (r w) -> p r w", w=W))
                return go

            def mk_conv(tname, half, ib):
                # Depthwise conv as 9 per-partition-scalar FMAs over shifted
                # views of the padded tile, on DVE. Output rows are split in
                # two image blocks (ib) so scores can chase conv progress.
                def go():
                    # TensorScalarPtr is not a legal GpSimd opcode on TRN2;
                    # all conv chains run on DVE.
                    eng = nc.vector
                    xp = xpad[(tname, half)]
                    dstt = kc[half] if tname == "k" else vc[half]
                    dst = dstt[:, ib * 512:(ib + 1) * 512].rearrange(
                        "p (y x) -> p y x", x=W)
                    y0 = 16 * ib
                    for tap in range(KSZ * KSZ):
                        dh, dw = divmod(tap, KSZ)
                        src = xp[:, y0 + DIL * dh:y0 + DIL * dh + 16,
                                 DIL * dw:DIL * dw + W]
                        if tap == 0:
                            eng.tensor_scalar(
                                out=dst, in0=src,
                                scalar1=wcol[:, half, 0:1],
                                scalar2=bias_c[:, half:half + 1],
                                op0=mybir.AluOpType.mult,
                                op1=mybir.AluOpType.add)
                        else:
                            eng.scalar_tensor_tensor(
                                out=dst, in0=src,
                                scalar=wcol[:, half, tap:tap + 1],
                                in1=dst,
                                op0=mybir.AluOpType.mult,
                                op1=mybir.AluOpType.add)
                return go

            def mk_vaug(kt):
                def go():
                    va3 = vaug[kt][:].rearrange("p (h x) -> p h x", x=HD + 1)
                    nc.vector.memset(va3[:, :, HD:HD + 1], 1.0)
                    for half in range(2):
                        pt = ppp.tile([128, 512], f32, tag="pp", name="pt")
                        nc.tensor.transpose(
                            pt[:, 0:128], vc[half][:, kt * 128:(kt + 1) * 128],
                            ident[:])
                        nc.vector.tensor_copy(
                            out=va3[:, 4 * half:4 * half + 4, 0:HD],
                            in_=pt[:, 0:128].rearrange("p (h d) -> p h d",
                                                       d=HD))
                return go

            dmas = []
            for kt in range(NKT):
                dmas.append(mk_dma("k", k_d, kt))
            for kt in range(NKT):
                dmas.append(mk_dma("q", q_d, kt))
            for kt in range(NKT):
                dmas.append(mk_dma("v", v_d, kt))
            # Prefix order minimizes time-to-first-score: k transposes feed
            # conv-k h0/ib0 ASAP, then the q columns the first unit reads.
            for kt in range(6):
                chunks.append(mk_trans("k", kt))
            chunks.append(mk_conv("k", 0, 0))
            for kt in range(4):
                chunks.append(mk_trans("q", kt))
            chunks.append(mk_trans("k", 6))
            chunks.append(mk_trans("k", 7))
            chunks.append(mk_conv("k", 0, 1))
            chunks.append(mk_conv("k", 1, 0))
            chunks.append(mk_conv("k", 1, 1))
            for kt in range(4, NKT):
                chunks.append(mk_trans("q", kt))
            n_a = len(chunks)  # prefix needed before scores can run
            for kt in range(NKT):
                chunks.append(mk_trans("v", kt))
            for half in range(2):
                for ib in range(2):
                    chunks.append(mk_conv("v", half, ib))
            for kt in range(NKT):
                chunks.append(mk_vaug(kt))
            return dmas, chunks[:n_a], chunks[n_a:]

        def attn_units(b):
            qc, kc, vaug = state[b]
            units = []

            def mk_unit(pair, qb):
                half, hl = divmod(pair, 2)
                q0 = qb * 512

                def go(fillers=(), pre_out=None, pre_av=None):
                    fillers = list(fillers)
                    n_f = len(fillers)
                    ptiles = []
                    for kt in range(NKT):
                        # evenly drain ALL assigned fillers across the kts
                        while len(fillers) > n_f * (NKT - 1 - kt) // NKT:
                            fillers.pop(0)()
                        if kt == 2 and pre_out is not None:
                            pre_out()
                        sc = sc_p.tile([128, 2, 512], f32, tag="sc", name="sc")
                        for j in range(2):
                            base = 64 * hl + 32 * j
                            nc.tensor.matmul(
                                out=sc[:, j, :],
                                lhsT=kc[half][base:base + 32,
                                              kt * 128:(kt + 1) * 128],
                                rhs=qc[half][base:base + 32, q0:q0 + 512],
                                start=True, stop=True,
                                tile_position=(base, 0))
                        p = p_p.tile([128, 2, 512], bf16, tag="p", name="p")
                        nc.scalar.activation(
                            out=p[:], in_=sc[:],
                            func=mybir.ActivationFunctionType.Exp,
                            scale=SCALE)
                        ptiles.append(p)
                    if pre_av is not None:
                        pre_av()
                    return mk_av(ptiles, pair, qb)
                return go

            def mk_av(ptiles, pair, qb):
                half, hl = divmod(pair, 2)

                def av():
                    acc = acc_p.tile([128, 512], f32, tag="acc", name="acc")
                    for kt in range(NKT):
                        for j in range(2):
                            hglob = half * 4 + hl * 2 + j
                            # j==0 widens lhsT to 64 cols so acc rows 33:64
                            # are written (defined junk from the next head's
                            # vaug cols) -- the later ot copy of rows 0:97
                            # must not read uninitialized PSUM. Same cost:
                            # matmul time is set by the 512 output columns.
                            w_l = 64 if j == 0 else HD + 1
                            nc.tensor.matmul(
                                out=acc[64 * j:64 * j + w_l, :],
                                lhsT=vaug[kt][:, (HD + 1) * hglob:
                                              (HD + 1) * hglob + w_l],
                                rhs=ptiles[kt][:, j, :],
                                start=(kt == 0), stop=(kt == NKT - 1))
                    h0 = half * 4 + hl * 2
                    ot = ot_p.tile([128, 512], f32, tag="ot", name="ot")
                    nc.vector.tensor_copy(out=ot[0:97, :], in_=acc[0:97, :])
                    def flush_out():
                        for u in range(4):
                          tp = ppp.tile([128, 512], f32, tag="pp", name="pt")
                          nc.tensor.transpose(
                              tp[:, 0:97], ot[0:97, u * 128:(u + 1) * 128],
                              ident[0:97, 0:97])
                          rc = small_p.tile([128, 2], f32, tag="rc", name="rc")
                          sums = bass.AP(tp.tensor, tp.offset + HD,
                                         [tp.ap[0], [64, 2]])
                          nc.vector.reciprocal(rc[:], sums)
                          otile = orow_p.tile([128, 2 * HD], f32, tag="orow",
                                              name="orow")
                          # otile[:, 32j+d] = tp[:, 64j+d] * rc[:, j]
                          src = bass.AP(tp.tensor, tp.offset,
                                        [tp.ap[0], [64, 2], [1, HD]])
                          rcb = bass.AP(rc.tensor, rc.offset,
                                        [rc.ap[0], [1, 2], [0, HD]])
                          nc.vector.tensor_tensor(
                              out=otile[:].rearrange("p (j d) -> p j d", d=HD),
                              in0=src, in1=rcb, op=mybir.AluOpType.mult)
                          nc.sync.dma_start(
                              out=out_d[b, (qb * 4 + u) * 128:
                                        (qb * 4 + u) * 128 + 128,
                                        HD * h0:HD * h0 + 2 * HD],
                              in_=otile[:])
                    return flush_out
                return av

            for pair in range(4):
                for qb in range(2):
                    units.append(mk_unit(pair, qb))
            return units

        # emission: all DMAs of batch 0 issued up front (async), then the
        # q/k/conv-k prefix (what scores need), then attn(b) interleaved
        # with [v path of b, DMAs + q/k prefix of b+1] as fillers.
        d0, a0, pending = prep_chunks(0)
        for ch in d0 + a0:
            ch()
        flush = None
        av_q = []  # AVs awaiting emission, deferred two units
        for b in range(BPC):
            units = attn_units(b)
            urgent = pending  # this batch's v path: needed by its own AVs
            if b + 1 < BPC:
                d1, a1, b1 = prep_chunks(b + 1)
                lazy = d1 + a1
                pending = b1
            else:
                lazy = []
                pending = []
            n_u = len(units)
            for i, unit in enumerate(units):
                # urgent chunks across the first 2 units (AVs are deferred
                # two units, so all v-path writes precede the first AV read);
                # lazy (next-batch) chunks across the remaining units.
                fl = []
                fl += urgent[i * len(urgent) // URG_U:
                             (i + 1) * len(urgent) // URG_U]
                if i >= URG_U:
                    j = i - URG_U
                    fl += lazy[j * len(lazy) // (n_u - URG_U):
                               (j + 1) * len(lazy) // (n_u - URG_U)]
                av = unit(fillers=fl, pre_out=flush, pre_av=None)
                av_q.append(av)
                if len(av_q) > AV_DEFER:
                    flush = av_q.pop(0)()
                else:
                    flush = None
        while av_q:
            fl = av_q.pop(0)()
            if flush is not None:
                flush()
            flush = fl
        if flush is not None:
            flush()
            flush = None


    return nc


def _get_nc():
    if "nc" not in _CACHE:
        import concourse.bass as bass
        import concourse.tile as tile
        from concourse import bacc, mybir

        nc = bacc.Bacc("TRN2", target_bir_lowering=False, debug=False)
        _build(nc, tile, bass, mybir)
        nc.compile()
        _CACHE["nc"] = nc
    return _CACHE["nc"]


def kernel(**inputs):
    q = np.ascontiguousarray(
        np.asarray(inputs["query"], dtype=np.float32).reshape(B, S, C))
    k = np.ascontiguousarray(
        np.asarray(inputs["key_in"], dtype=np.float32).reshape(B, S, C))
    v = np.ascontiguousarray(
        np.asarray(inputs["value"], dtype=np.float32).reshape(B, S, C))
    ck = np.ascontiguousarray(
        np.asarray(inputs["conv_kernel"], dtype=np.float32).reshape(
            KSZ * KSZ, C))
    cb = np.ascontiguousarray(
        np.asarray(inputs["conv_bias"], dtype=np.float32).reshape(C))

    in_maps = []
    for i in range(NCORES):
        lo, hi = i * BPC, (i + 1) * BPC
        in_maps.append({
            "query": np.ascontiguousarray(q[lo:hi]),
            "key_in": np.ascontiguousarray(k[lo:hi]),
            "value": np.ascontiguousarray(v[lo:hi]),
            "conv_kernel": ck,
            "conv_bias": cb,
        })

    from concourse.bass_utils import run_bass_kernel_spmd

    nc = _get_nc()
    res = run_bass_kernel_spmd(
        nc, in_maps, core_ids=list(range(NCORES)),
        **_CACHE.get("run_kwargs", {}),
    )
    _CACHE["last_result"] = res
    out = np.concatenate([r["out"] for r in res.results], axis=0)
    return out.reshape(B, H, W, C)


# revision 17
# speedup vs baseline: 1.7566x; 1.7566x over previous
"""v5: v2 with the depthwise conv moved to the Tensor engine."""

import numpy as np

B, H, W, C = 16, 32, 32, 256
HEADS = 8
HD = C // HEADS            # 32
KSZ, DIL = 3, 2
SCALE = float(HD) ** -0.5
NCORES = 8
BPC = B // NCORES          # batches per core
S = H * W                  # 1024
NKT = S // 128             # 8 k/s tiles
AV_DEFER = 2               # units between scores and their AV emission
URG_U = 2                  # units that absorb the current batch's v-path
import math as _math
NAPX = 0                   # kt tiles per unit whose exp runs on DVE
EXPA = (float(32) ** -0.5) * 128.0 / _math.log(2.0)
EXPB = 16256.0 - 8.0

_CACHE = {}


def _build(nc, tile, bass, mybir, repeat=None, parts="all"):
    from contextlib import ExitStack
    from concourse.masks import make_identity

    f32 = mybir.dt.float32
    bf16 = mybir.dt.bfloat16

    q_d = nc.dram_tensor("query", [BPC, S, C], f32, kind="ExternalInput")
    k_d = nc.dram_tensor("key_in", [BPC, S, C], f32, kind="ExternalInput")
    v_d = nc.dram_tensor("value", [BPC, S, C], f32, kind="ExternalInput")
    ck_d = nc.dram_tensor("conv_kernel", [KSZ * KSZ, C], f32, kind="ExternalInput")
    cb_d = nc.dram_tensor("conv_bias", [C], f32, kind="ExternalInput")
    out_d = nc.dram_tensor("out", [BPC, S, C], f32, kind="ExternalOutput")

    HP = H + 2 * DIL  # padded rows: 36
    WP = W + 2 * DIL  # padded cols: 36

    with ExitStack() as ctx:
        tc = ctx.enter_context(tile.TileContext(nc))
        const = ctx.enter_context(tc.tile_pool(name="const", bufs=1))
        sin = ctx.enter_context(tc.tile_pool(name="sin", bufs=32))
        xpad_p = ctx.enter_context(tc.tile_pool(name="xpad", bufs=8))
        qc_p = ctx.enter_context(tc.tile_pool(name="qc", bufs=4))
        kc_p = ctx.enter_context(tc.tile_pool(name="kc", bufs=4))
        vc_p = ctx.enter_context(tc.tile_pool(name="vc", bufs=4))
        vaug_p = ctx.enter_context(tc.tile_pool(name="vaug", bufs=16))
        p_p = ctx.enter_context(tc.tile_pool(name="pp", bufs=40))
        ot_p = ctx.enter_context(tc.tile_pool(name="ot", bufs=4))
        orow_p = ctx.enter_context(tc.tile_pool(name="orow", bufs=16))
        small_p = ctx.enter_context(tc.tile_pool(name="small", bufs=8))
        # PSUM: trans pool (2 banks) + scores (2x2 banks) + accum (2 banks)
        ppp = ctx.enter_context(tc.tile_pool(name="ppp", bufs=2, space="PSUM"))
        sc_p = ctx.enter_context(tc.tile_pool(name="scp", bufs=2, space="PSUM"))
        acc_p = ctx.enter_context(tc.tile_pool(name="accp", bufs=2, space="PSUM"))

        # ---- constants ----
        ident = const.tile([128, 128], f32)
        make_identity(nc, ident[:])

        # conv weights as per-partition scalars: wcol[c, half, tap]
        wcol = const.tile([128, 2, KSZ * KSZ], f32)
        for half in range(2):
            nc.gpsimd.dma_start(
                out=wcol[:, half],
                in_=bass.AP(ck_d, half * 128, [[1, 128], [C, KSZ * KSZ]]),
            )

        bias_c = const.tile([128, 2], f32)
        for half in range(2):
            nc.gpsimd.dma_start(
                out=bias_c[:, half:half + 1],
                in_=bass.AP(cb_d, half * 128, [[1, 128], [1, 1]]),
            )

        # diagonal weight matrices for the PE conv: diagw[c, half, tap, c']
        diagw = const.tile([128, 2, KSZ * KSZ, 128], bf16)
        for half in range(2):
            for tap in range(KSZ * KSZ):
                nc.vector.tensor_scalar(
                    out=diagw[:, half, tap], in0=ident[:],
                    scalar1=wcol[:, half, tap:tap + 1], scalar2=None,
                    op0=mybir.AluOpType.mult)

        rep_ctx = tc.For_i(0, repeat, 1) if repeat else None
        if rep_ctx is not None:
            ctx.enter_context(rep_ctx)

        state = {}

        def prep_chunks(b):
            """Emit-able closures for batch b's prep; fills state[b]."""
            qc = [qc_p.tile([128, S], bf16, tag="qc", name="qc") for _ in range(2)]
            kc = [kc_p.tile([128, S], bf16, tag="kc", name="kc") for _ in range(2)]
            vc = [vc_p.tile([128, S], f32, tag="vc", name="vc") for _ in range(2)]
            vaug = [vaug_p.tile([128, HEADS * (HD + 1)], bf16, tag="va",
                                name="va") for _ in range(NKT)]
            xpad = {}
            state[b] = (qc, kc, vaug)
            chunks = []

            def mk_xpad():
                for tname in ("k", "v"):
                    for half in range(2):
                        xp = xpad_p.tile([128, HP, WP], bf16, tag="xpad",
                                         name="xpad")
                        # zero only the padding ring; interior is overwritten
                        eng = nc.gpsimd
                        eng.memset(xp[:, 0:DIL, :], 0.0)
                        eng.memset(xp[:, HP - DIL:HP, :], 0.0)
                        eng.memset(xp[:, DIL:HP - DIL, 0:DIL], 0.0)
                        eng.memset(xp[:, DIL:HP - DIL, WP - DIL:WP], 0.0)
                        xpad[(tname, half)] = xp
            chunks.append(mk_xpad)

            st_tiles = {}

            def mk_dma(tname, dram, kt):
                def go():
                    st = sin.tile([128, C], f32, tag="sin", name="st")
                    nc.sync.dma_start(
                        out=st[:], in_=dram[b, kt * 128:(kt + 1) * 128, :])
                    st_tiles[(tname, kt)] = st
                return go

            def mk_trans(tname, kt):
                def go():
                    st = st_tiles[(tname, kt)]
                    for half in range(2):
                        pt = ppp.tile([128, 512], f32, tag="pp", name="pt")
                        nc.tensor.transpose(
                            pt[:, 0:128], st[:, 128 * half:128 * half + 128],
                            ident[:])
                        if tname == "q":
                            nc.vector.tensor_copy(
                                out=qc[half][:, kt * 128:(kt + 1) * 128],
                                in_=pt[:, 0:128])
                        else:
                            dst = xpad[(tname, half)][
                                :, DIL + 4 * kt:DIL + 4 * kt + 4, DIL:DIL + W]
                            nc.vector.tensor_copy(
                                out=dst,
                                in_=pt[:, 0:128].rearrange(
                                    "p (r w) -> p r w", w=W))
                return go

            def mk_conv(tname, half, ib):
                # Depthwise conv as 9 accumulating diag-weight matmuls on the
                # Tensor engine over shifted views of the padded tile. PSUM
                # from the AV acc pool; evac fuses the conv bias add on DVE.
                def go():
                    xp = xpad[(tname, half)]
                    cp = acc_p.tile([128, 16, W], f32, tag="acc", name="cv")
                    y0 = 16 * ib
                    for tap in range(KSZ * KSZ):
                        dh, dw = divmod(tap, KSZ)
                        nc.tensor.matmul(
                            out=cp[:],
                            lhsT=diagw[:, half, tap],
                            rhs=xp[:, y0 + DIL * dh:y0 + DIL * dh + 16,
                                   DIL * dw:DIL * dw + W],
                            start=(tap == 0), stop=(tap == KSZ * KSZ - 1))
                    dstt = kc[half] if tname == "k" else vc[half]
                    nc.vector.tensor_scalar(
                        out=dstt[:, ib * 512:(ib + 1) * 512],
                        in0=cp[:].rearrange("p y x -> p (y x)"),
                        scalar1=bias_c[:, half:half + 1], scalar2=None,
                        op0=mybir.AluOpType.add)
                return go

            def mk_vaug(kt):
                def go():
                    va3 = vaug[kt][:].rearrange("p (h x) -> p h x", x=HD + 1)
                    nc.vector.memset(va3[:, :, HD:HD + 1], 1.0)
                    for half in range(2):
                        pt = ppp.tile([128, 512], f32, tag="pp", name="pt")
                        nc.tensor.transpose(
                            pt[:, 0:128], vc[half][:, kt * 128:(kt + 1) * 128],
                            ident[:])
                        nc.vector.tensor_copy(
                            out=va3[:, 4 * half:4 * half + 4, 0:HD],
                            in_=pt[:, 0:128].rearrange("p (h d) -> p h d",
                                                       d=HD))
                return go

            dmas = []
            for kt in range(NKT):
                dmas.append(mk_dma("k", k_d, kt))
            for kt in range(NKT):
                dmas.append(mk_dma("q", q_d, kt))
            for kt in range(NKT):
                dmas.append(mk_dma("v", v_d, kt))
            # Prefix order minimizes time-to-first-score: k transposes feed
            # conv-k h0/ib0 ASAP, then the q columns the first unit reads.
            for kt in range(6):
                chunks.append(mk_trans("k", kt))
            chunks.append(mk_conv("k", 0, 0))
            for kt in range(4):
                chunks.append(mk_trans("q", kt))
            chunks.append(mk_trans("k", 6))
            chunks.append(mk_trans("k", 7))
            chunks.append(mk_conv("k", 0, 1))
            chunks.append(mk_conv("k", 1, 0))
            chunks.append(mk_conv("k", 1, 1))
            for kt in range(4, NKT):
                chunks.append(mk_trans("q", kt))
            n_a = len(chunks)  # prefix needed before scores can run
            for kt in range(NKT):
                chunks.append(mk_trans("v", kt))
            for half in range(2):
                for ib in range(2):
                    chunks.append(mk_conv("v", half, ib))
            for kt in range(NKT):
                chunks.append(mk_vaug(kt))
            return dmas, chunks[:n_a], chunks[n_a:]

        def attn_units(b):
            qc, kc, vaug = state[b]
            units = []

            def mk_unit(pair, qb):
                half, hl = divmod(pair, 2)
                q0 = qb * 512

                def go(fillers=(), pre_out=None, pre_av=None):
                    fillers = list(fillers)
                    n_f = len(fillers)
                    ptiles = []
                    for kt in range(NKT):
                        # evenly drain ALL assigned fillers across the kts
                        while len(fillers) > n_f * (NKT - 1 - kt) // NKT:
                            fillers.pop(0)()
                        if kt == 2 and pre_out is not None:
                            pre_out()
                        sc = sc_p.tile([128, 2, 512], f32, tag="sc", name="sc")
                        for j in range(2):
                            base = 64 * hl + 32 * j
                            nc.tensor.matmul(
                                out=sc[:, j, :],
                                lhsT=kc[half][base:base + 32,
                                              kt * 128:(kt + 1) * 128],
                                rhs=qc[half][base:base + 32, q0:q0 + 512],
                                start=True, stop=True,
                                tile_position=(base, 0))
                        p = p_p.tile([128, 2, 512], bf16, tag="p", name="p")
                        if kt >= NKT - NAPX:
                            nc.vector.tensor_scalar(
                                out=p[:].bitcast(mybir.dt.int16), in0=sc[:],
                                scalar1=EXPA, scalar2=EXPB,
                                op0=mybir.AluOpType.mult,
                                op1=mybir.AluOpType.add)
                        else:
                            nc.scalar.activation(
                                out=p[:], in_=sc[:],
                                func=mybir.ActivationFunctionType.Exp,
                                scale=SCALE)
                        ptiles.append(p)
                    if pre_av is not None:
                        pre_av()
                    return mk_av(ptiles, pair, qb)
                return go

            def mk_av(ptiles, pair, qb):
                half, hl = divmod(pair, 2)

                def av():
                    acc = acc_p.tile([128, 512], f32, tag="acc", name="acc")
                    for kt in range(NKT):
                        for j in range(2):
                            hglob = half * 4 + hl * 2 + j
                            # j==0 widens lhsT to 64 cols so acc rows 33:64
                            # are written (defined junk from the next head's
                            # vaug cols) -- the later ot copy of rows 0:97
                            # must not read uninitialized PSUM. Same cost:
                            # matmul time is set by the 512 output columns.
                            w_l = 64 if j == 0 else HD + 1
                            nc.tensor.matmul(
                                out=acc[64 * j:64 * j + w_l, :],
                                lhsT=vaug[kt][:, (HD + 1) * hglob:
                                              (HD + 1) * hglob + w_l],
                                rhs=ptiles[kt][:, j, :],
                                start=(kt == 0), stop=(kt == NKT - 1))
                    h0 = half * 4 + hl * 2
                    ot = ot_p.tile([128, 512], f32, tag="ot", name="ot")
                    nc.vector.tensor_copy(out=ot[0:97, :], in_=acc[0:97, :])
                    def flush_out():
                        for u in range(4):
                          tp = ppp.tile([128, 512], f32, tag="pp", name="pt")
                          nc.tensor.transpose(
                              tp[:, 0:97], ot[0:97, u * 128:(u + 1) * 128],
                              ident[0:97, 0:97])
                          rc = small_p.tile([128, 2], f32, tag="rc", name="rc")
                          sums = bass.AP(tp.tensor, tp.offset + HD,
                                         [tp.ap[0], [64, 2]])
                          nc.vector.reciprocal(rc[:], sums)
                          otile = orow_p.tile([128, 2 * HD], f32, tag="orow",
                                              name="orow")
                          # otile[:, 32j+d] = tp[:, 64j+d] * rc[:, j]
                          src = bass.AP(tp.tensor, tp.offset,
                                        [tp.ap[0], [64, 2], [1, HD]])
                          rcb = bass.AP(rc.tensor, rc.offset,
                                        [rc.ap[0], [1, 2], [0, HD]])
                          nc.vector.tensor_tensor(
                              out=otile[:].rearrange("p (j d) -> p j d", d=HD),
                              in0=src, in1=rcb, op=mybir.AluOpType.mult)
                          nc.sync.dma_start(
                              out=out_d[b, (qb * 4 + u) * 128:
                                        (qb * 4 + u) * 128 + 128,
                                        HD * h0:HD * h0 + 2 * HD],
                              in_=otile[:])
                    return flush_out
                return av

            for pair in range(4):
                for qb in range(2):
                    units.append(mk_unit(pair, qb))
            return units

        # emission: all DMAs of batch 0 issued up front (async), then the
        # q/k/conv-k prefix (what scores need), then attn(b) interleaved
        # with [v path of b, DMAs + q/k prefix of b+1] as fillers.
        d0, a0, pending = prep_chunks(0)
        for ch in d0 + a0:
            ch()
        flush = None
        av_q = []  # AVs awaiting emission, deferred two units
        for b in range(BPC):
            units = attn_units(b)
            urgent = pending  # this batch's v path: needed by its own AVs
            if b + 1 < BPC:
                d1, a1, b1 = prep_chunks(b + 1)
                lazy = d1 + a1
                pending = b1
            else:
                lazy = []
                pending = []
            n_u = len(units)
            for i, unit in enumerate(units):
                # urgent chunks across the first 2 units (AVs are deferred
                # two units, so all v-path writes precede the first AV read);
                # lazy (next-batch) chunks across the remaining units.
                fl = []
                fl += urgent[i * len(urgent) // URG_U:
                             (i + 1) * len(urgent) // URG_U]
                if i >= URG_U:
                    j = i - URG_U
                    fl += lazy[j * len(lazy) // (n_u - URG_U):
                               (j + 1) * len(lazy) // (n_u - URG_U)]
                av = unit(fillers=fl, pre_out=flush, pre_av=None)
                av_q.append(av)
                if len(av_q) > AV_DEFER:
                    flush = av_q.pop(0)()
                else:
                    flush = None
        while av_q:
            fl = av_q.pop(0)()
            if flush is not None:
                flush()
            flush = fl
        if flush is not None:
            flush()
            flush = None


    return nc


def _get_nc():
    if "nc" not in _CACHE:
        import concourse.bass as bass
        import concourse.tile as tile
        from concourse import bacc, mybir

        nc = bacc.Bacc("TRN2", target_bir_lowering=False, debug=False)
        _build(nc, tile, bass, mybir)
        nc.compile()
        _CACHE["nc"] = nc
    return _CACHE["nc"]


def kernel(**inputs):
    q = np.ascontiguousarray(
        np.asarray(inputs["query"], dtype=np.float32).reshape(B, S, C))
    k = np.ascontiguousarray(
        np.asarray(inputs["key_in"], dtype=np.float32).reshape(B, S, C))
    v = np.ascontiguousarray(
        np.asarray(inputs["value"], dtype=np.float32).reshape(B, S, C))
    ck = np.ascontiguousarray(
        np.asarray(inputs["conv_kernel"], dtype=np.float32).reshape(
            KSZ * KSZ, C))
    cb = np.ascontiguousarray(
        np.asarray(inputs["conv_bias"], dtype=np.float32).reshape(C))

    in_maps = []
    for i in range(NCORES):
        lo, hi = i * BPC, (i + 1) * BPC
        in_maps.append({
            "query": np.ascontiguousarray(q[lo:hi]),
            "key_in": np.ascontiguousarray(k[lo:hi]),
            "value": np.ascontiguousarray(v[lo:hi]),
            "conv_kernel": ck,
            "conv_bias": cb,
        })

    from concourse.bass_utils import run_bass_kernel_spmd

    nc = _get_nc()
    res = run_bass_kernel_spmd(
        nc, in_maps, core_ids=list(range(NCORES)),
        **_CACHE.get("run_kwargs", {}),
    )
    _CACHE["last_result"] = res
    out = np.concatenate([r["out"] for r in res.results], axis=0)
    return out.reshape(B, H, W, C)
